# revision 69
# baseline (speedup 1.0000x reference)
"""Causal self-attention with RoPE on 8 Trainium2 NeuronCores (Bass/Tile).

Sharding: 8 cores = 2 batch elements x 4 head-groups (4 heads each), no
collectives. Each core computes QKV for its heads from a host-pretransposed
x^T, applies RoPE, runs causally-trimmed flash-style attention, and emits a
partial output projection against its w_proj row-slice; the host sums 4
partials per batch element.

v3 design (from TimelineSim engine-balance analysis):
- The exp stream on ACT (~1040ns/step) is the schedule backbone; all other
  PE work (QKV chains of the next chunk, V tiles, PV groups of the previous
  chunk, transposes, projection tiles) is emitted as paced "fillers" between
  S steps so PE never drains while ACT is the local rate limiter.
- PV uses the [q-tile, hd] layout: lhsT = eS^T tile (k x q), rhs = v (k x
  hd): N=65-ish per (q-tile, k-tile) at full 128x128 array utilization --
  half the cycles of the [hd, q] layout. Softmax denominators become
  per-partition scalars (ones-column matmuls into a dedicated PSUM bank),
  normalization is a DVE reciprocal + broadcast multiply, and a per-chunk
  PE transpose (through the psY bank's dead window) restores the [hd, t]
  layout the projection needs.
- Causal masking: exp of the diagonal tile is multiplied by a 0/1 triangle
  mask on DVE (no PE mask matmuls).
- PSUM (8 banks): S ring 2x2 + psY 2x1 + denom 1 + QKV/proj shared slot 1.
- eS tiles are materialized in SBUF per chunk (j<4 double-buffered) so the
  PV of chunk cq overlaps the S stream of chunk cq+1.
"""
import os

import numpy as np

import concourse.bass as bass
import concourse.mybir as mybir
import concourse.tile as tile
from concourse import bacc
from concourse.bass_utils import run_bass_kernel_spmd

# Problem shape (hardcoded per harness contract).
B, T, C, NH = 2, 2048, 1024, 16
HD = C // NH          # 64
HPC = NH // 4         # 4 heads per core
N_CORES = 8
ROPE_BASE = 10000.0
NEG = -1.0e30

F32 = mybir.dt.float32
BF16 = mybir.dt.bfloat16

_CACHE = {}


def _rope_tables_T():
    """cos/sin tables transposed to [HD, T], duplicated to 128 partitions
    (two 64-row head blocks). The rotate-half sign lives in the rot
    matrix, not the sin table."""
    inv_freq = 1.0 / (ROPE_BASE ** (np.arange(0, HD, 2, dtype=np.float32) / HD))
    t = np.arange(T, dtype=np.float32)
    freqs = np.outer(t, inv_freq).astype(np.float32)      # [T, 32]
    emb = np.concatenate([freqs, freqs], axis=-1)         # [T, 64]
    cosT = np.cos(emb).T.astype(np.float32)               # [64, T]
    sinT = np.sin(emb).T.astype(np.float32)
    cos2 = np.concatenate([cosT, cosT], axis=0)           # [128, T]
    sin2 = np.concatenate([sinT, sinT], axis=0)
    return np.ascontiguousarray(cos2), np.ascontiguousarray(sin2)


def _rot_matrix():
    """rot128 so that (rot128.T @ qT) = rotate_half(q)^T per 64-row head
    block: out[d] = -in[d+32] for d<32, in[d-32] for d>=32."""
    r = np.zeros((64, 64), dtype=np.float32)
    for d in range(32):
        r[d, d + 32] = -1.0
        r[d + 32, d] = 1.0
    z = np.zeros_like(r)
    rot = np.block([[r, z], [z, r]])          # [128, 128]
    return np.ascontiguousarray(rot.T)        # lhsT layout


def _mask_tile():
    """mask01 [128, 128] (bf16): 1.0 where q_local >= k_local else 0.0.
    Applied multiplicatively to exp(S) diagonal tiles on DVE."""
    import ml_dtypes
    k_l = np.arange(128)[:, None]
    q_l = np.arange(128)[None, :]
    m = np.where(q_l >= k_l, 1.0, 0.0).astype(ml_dtypes.bfloat16)
    return m


def build_nc():
    nc = bacc.Bacc(None, target_bir_lowering=False)

    # xw packs [wqk | xT] per row so one DMA fetches a c-block of both
    xw = nc.dram_tensor("xw", [C, 8 * HD + T], BF16, kind="ExternalInput")
    wv = nc.dram_tensor("wv", [C, 4 * HD], BF16, kind="ExternalInput")
    wp = nc.dram_tensor("wp", [4 * HD, C], BF16, kind="ExternalInput")
    cossin_d = nc.dram_tensor("cossin", [128, 2, T], BF16,
                              kind="ExternalInput")
    consts3_d = nc.dram_tensor("consts3", [128, 3, 128], BF16,
                               kind="ExternalInput")
    outp = nc.dram_tensor("outp", [T, C], BF16, kind="ExternalOutput")

    NT = T // 128    # 16 k-tiles
    NQ = T // 512    # 4 q-chunks
    FILLER_NS = int(os.environ.get("KERNEL_FILLER_NS", "650"))
    LEAD = int(os.environ.get("KERNEL_LEAD", "6"))

    with tile.TileContext(nc) as tc:
        with (
            tc.tile_pool(name="persist", bufs=1) as persist,
            tc.tile_pool(name="consts", bufs=1) as consts,
            tc.tile_pool(name="psp", bufs=2, space="PSUM") as psp,
            tc.tile_pool(name="work", bufs=3) as work,
        ):
            # ---- persistent tiles ----
            qk_packed = [
                persist.tile([128, T], BF16, name=f"qkp{w}", tag=f"qkp{w}")
                for w in range(4)
            ]
            vtil = persist.tile([128, NT, 4, HD + 1], BF16, name="vtil")
            ynorm = [
                persist.tile([128, T], BF16, name=f"ynorm{g}", tag=f"ynorm{g}")
                for g in range(2)
            ]
            es_main = [
                persist.tile([128, NT, 2, 512], BF16, name=f"esm{g}",
                             tag=f"esm{g}")
                for g in range(2)
            ]
            es_alt = [
                persist.tile([128, 4, 2, 512], BF16, name=f"esa{g}",
                             tag=f"esa{g}")
                for g in range(2)
            ]
            consts3_sb = consts.tile([128, 3, 128], BF16, name="consts3_sb")
            rot_sb = consts3_sb[:, 0, :]
            ident_sb = consts3_sb[:, 1, :]
            mask01_sb = consts3_sb[:, 2, :]
            wp_sb = consts.tile([128, 2, C], BF16, name="wp_sb")
            xw_sb = persist.tile([128, C // 128, 512 + T], BF16,
                                 name="xw_sb")
            wqk_sb = xw_sb[:, :, 0:512]
            xT_sb = xw_sb[:, :, 512:512 + T]
            cossin_sb = persist.tile([128, 2, T], BF16, name="cossin_sb")
            cos2_sb = cossin_sb[:, 0, :]
            sin2_sb = cossin_sb[:, 1, :]
            wv_sb = persist.tile([128, C // 128, 4 * HD], BF16, name="wv_sb")

            xw_r = xw.rearrange("(co p) t -> p co t", p=128)

            # Each DMA's issue holds HWDGE ~625ns exclusively, and small
            # transfers don't hide the next issue, so the head is
            # issue-rate-bound: use few, large DMAs ordered by first
            # consumption (tables are host-packed into single tensors).
            for c in range(2):
                nc.sync.dma_start(out=xw_sb[:, c:c + 1, 0:1024],
                                  in_=xw_r[:, c:c + 1, 0:1024])
            nc.sync.dma_start(out=consts3_sb, in_=consts3_d[:, :, :])
            nc.sync.dma_start(out=xw_sb[:, 2:5, 0:1024],
                              in_=xw_r[:, 2:5, 0:1024])
            nc.sync.dma_start(out=xw_sb[:, 5:8, 0:1024],
                              in_=xw_r[:, 5:8, 0:1024])
            nc.sync.dma_start(out=cossin_sb[:, :, 0:512],
                              in_=cossin_d[:, :, 0:512])
            nc.sync.dma_start(
                out=wv_sb, in_=wv.rearrange("(co p) n -> p co n", p=128))
            nc.sync.dma_start(out=xw_sb[:, :, 1024:1536],
                              in_=xw_r[:, :, 1024:1536])
            nc.sync.dma_start(out=cossin_sb[:, :, 512:1024],
                              in_=cossin_d[:, :, 512:1024])
            nc.sync.dma_start(out=xw_sb[:, :, 1536:2048],
                              in_=xw_r[:, :, 1536:2048])
            nc.sync.dma_start(out=cossin_sb[:, :, 1024:T],
                              in_=cossin_d[:, :, 1024:T])
            nc.sync.dma_start(
                out=wp_sb, in_=wp.rearrange("(gg p) n -> p gg n", p=128))
            nc.sync.dma_start(out=xw_sb[:, :, 2048:2560],
                              in_=xw_r[:, :, 2048:2560])

            ones64 = persist.tile([128, NT * 4], BF16, name="ones64")
            nc.vector.memset(ones64, 1.0)
            nc.vector.tensor_copy(
                out=vtil[:, :, :, HD:HD + 1],
                in_=ones64.rearrange("p (a b) -> p a b", a=NT).unsqueeze(-1),
            )
            inv_sqrt_hd = float(1.0 / np.sqrt(HD))

            def es_ap(g, cq, j):
                """eS store for (chunk, k-tile): j<4 double-buffered by
                chunk parity so S(cq+1) can overlap PV(cq)."""
                if j < 4 and (cq % 2) == 1:
                    return es_alt[g][:, j]
                return es_main[g][:, j]

            # ---------- emission pieces ----------
            def chain_closures(w, cq):
                """QKV chain for wqk column-block w over chunk cq + RoPE.
                Returns (pe_cost_ns, closure) list; the accumulator lives in
                the shared 1-bank bigA slot, rotate-half reuses it after the
                raw copy frees it."""
                sl = slice(cq * 512, (cq + 1) * 512)
                st = {}
                out = []

                def mk(c):
                    def f():
                        if c == 0:
                            st["ps"] = psp.tile([128, 512], F32, name="psqkv",
                                                tag="bigA", bufs=1)
                        nc.tensor.matmul(
                            st["ps"],
                            lhsT=wqk_sb[:, c, w * 128:(w + 1) * 128],
                            rhs=xT_sb[:, c, sl],
                            start=(c == 0), stop=(c == C // 128 - 1),
                        )
                    return f
                for c in range(C // 128):
                    out.append((220, mk(c)))

                def raw_f():
                    raw = work.tile([128, 512], BF16, name="raw", tag="raw",
                                    bufs=4)
                    st["raw"] = raw
                    nc.vector.tensor_copy(out=raw, in_=st["ps"])
                out.append((0, raw_f))

                def rot_f():
                    psr = psp.tile([128, 512], F32, name="psr",
                                   tag="bigA", bufs=1)
                    nc.tensor.matmul(psr, lhsT=rot_sb, rhs=st["raw"],
                                     start=True, stop=True)
                    tmp = work.tile([128, 512], F32, name="tmp", tag="tmp",
                                    bufs=4)
                    nc.vector.tensor_mul(tmp, psr, sin2_sb[:, sl])
                    cosq = work.tile([128, 512], BF16, name="cosq",
                                     tag="cosq", bufs=4)
                    if w in (0, 2):
                        # g=0's qk blocks gate the next pass's first S
                        # steps; keep their epilogue off the slow GPSIMD.
                        nc.vector.tensor_mul(cosq, st["raw"], cos2_sb[:, sl])
                        nc.vector.tensor_add(qk_packed[w][:, sl], cosq, tmp)
                    else:
                        nc.gpsimd.tensor_mul(cosq, st["raw"], cos2_sb[:, sl])
                        nc.gpsimd.tensor_add(qk_packed[w][:, sl], cosq, tmp)
                out.append((220, rot_f))
                return out

            def v_closures(tt):
                """V tile tt (t-major) through the shared bigA slot."""
                st = {}
                out = []

                def mk(c):
                    def f():
                        if c == 0:
                            st["ps"] = psp.tile([128, 4 * HD], F32,
                                                name="psv", tag="bigA",
                                                bufs=1)
                        nc.tensor.matmul(
                            st["ps"],
                            lhsT=xT_sb[:, c, tt * 128:(tt + 1) * 128],
                            rhs=wv_sb[:, c, :],
                            start=(c == 0), stop=(c == C // 128 - 1),
                        )
                    return f
                for c in range(C // 128):
                    out.append((110, mk(c)))

                def epi():
                    nc.vector.tensor_copy(
                        out=vtil[:, tt, :, 0:HD],
                        in_=st["ps"].rearrange("p (h d) -> p h d", h=4),
                    )
                out.append((0, epi))
                return out

            def s_step(g, cq, j):
                """S^T matmuls (both heads of pair g) for k-tile j, exp on
                ACT into the eS store, 0/1 mask multiply on DVE for the
                diagonal tile."""
                off = max(0, (j - 4 * cq) * 128)
                F = 512 - off
                qlo = cq * 512 + off
                psS = psp.tile([128, 2, 512], F32, name="psS", tag="psS",
                               bufs=2)
                for hh in range(2):
                    poff = 64 * hh
                    nc.tensor.matmul(
                        psS[:, hh, 0:F],
                        lhsT=qk_packed[2 + g][poff:poff + 64,
                                              j * 128:(j + 1) * 128],
                        rhs=qk_packed[g][poff:poff + 64, qlo:qlo + F],
                        start=True, stop=True,
                    )
                est = es_ap(g, cq, j)
                nc.scalar.activation(
                    out=est[:, :, off:512], in_=psS[:, :, 0:F],
                    func=mybir.ActivationFunctionType.Exp,
                    scale=inv_sqrt_hd,
                )
                if j >= 4 * cq:
                    nc.vector.tensor_tensor(
                        out=est[:, :, off:off + 128],
                        in0=est[:, :, off:off + 128],
                        in1=mask01_sb.unsqueeze(1).to_broadcast([128, 2, 128]),
                        op=mybir.AluOpType.mult,
                    )

            psY_t = {}
            psD_t = {}

            def pv_group(g, cq, qt, hh):
                """One PV accumulation group: q-tile qt of pair-g head hh,
                contracting over k-tiles 0..4cq+qt into psY[q, hd] plus the
                ones-column denominator into the psD bank."""
                njt = 4 * cq + qt + 1

                def f():
                    if psY_t.get(g, (None, -1))[1] != cq:
                        psY_t[g] = (psp.tile([128, 2, 4, HD], F32,
                                             name=f"psY{g}", tag=f"psY{g}",
                                             bufs=1), cq)
                    if psD_t.get("cq", -1) != cq:
                        psD_t["t"] = psp.tile([128, 2, 2, 4, 1], F32,
                                              name="psD", tag="psD", bufs=1)
                        psD_t["cq"] = cq
                    psY = psY_t[g][0]
                    psD = psD_t["t"]
                    h = 2 * g + hh
                    for j in range(njt):
                        lhsT = es_ap(g, cq, j)[:, hh, qt * 128:(qt + 1) * 128]
                        nc.tensor.matmul(
                            psY[:, hh, qt, :], lhsT=lhsT,
                            rhs=vtil[:, j, h, 0:HD],
                            start=(j == 0), stop=(j == njt - 1),
                        )
                        nc.tensor.matmul(
                            psD[:, g, hh, qt, :], lhsT=lhsT,
                            rhs=vtil[:, j, h, HD:HD + 1],
                            start=(j == 0), stop=(j == njt - 1),
                        )
                return (njt * 2 * 30, f)

            ysb_t = {}

            def norm_cl(g, cq, qt):
                """Normalize q-tile qt of pair g: per-partition reciprocal
                of the denominator column, broadcast multiply out of PSUM."""
                def f():
                    psY = psY_t[g][0]
                    psD = psD_t["t"]
                    rec = work.tile([128, 2, 1], F32, name="rec", tag="rec",
                                    bufs=6)
                    nc.vector.reciprocal(rec, psD[:, g, :, qt, :])
                    ysb = work.tile([128, 2, HD], BF16, name="ysb",
                                    tag="ysb", bufs=10)
                    nc.vector.tensor_tensor(
                        out=ysb, in0=psY[:, :, qt, :],
                        in1=rec.to_broadcast([128, 2, HD]),
                        op=mybir.AluOpType.mult,
                    )
                    ysb_t[(g, qt)] = ysb
                return (0, f)

            def transp_cl(g, cq):
                """Transpose the chunk's 4 normalized q-tiles of pair g back
                to [hd, t] through the psY bank's dead window, then one ACT
                copy into ynorm."""
                def f():
                    psT = psp.tile([128, 4, 128], F32, name=f"psT{g}",
                                   tag=f"psY{g}", bufs=1)
                    for qt in range(4):
                        nc.tensor.matmul(
                            psT[:, qt, :],
                            lhsT=ysb_t[(g, qt)].rearrange("p a b -> p (a b)"),
                            rhs=ident_sb,
                            start=True, stop=True,
                        )
                    if cq == NQ - 1:
                        # tail: DVE is the drain bottleneck, ACT is idle
                        nc.scalar.copy(
                            out=ynorm[g][:, cq * 512:(cq + 1) * 512],
                            in_=psT.rearrange("p a b -> p (a b)"),
                        )
                    else:
                        nc.vector.tensor_copy(
                            out=ynorm[g][:, cq * 512:(cq + 1) * 512],
                            in_=psT.rearrange("p a b -> p (a b)"),
                        )
                return (4 * 55, f)

            def proj_cl(cq, qt, half, ring="bigA"):
                """Half-width projection of t-tile 4cq+qt; accumulator from
                the shared bigA slot (or the freed psS ring in the final
                drain); PSUM->SBUF copy alternates ACT/DVE. Both halves of a
                t-tile share one ost tile and one output DMA (HWDGE issue
                costs ~625ns each). The filler cost is inflated past one
                grant so consecutive projections are spaced by at least one
                S step."""
                tt = 4 * cq + qt

                def f():
                    if ring == "psS":
                        pso = psp.tile([128, 2, 512], F32, name="psoS",
                                       tag="psS", bufs=2)[:, 0, :]
                    else:
                        pso = psp.tile([128, 512], F32, name="pso",
                                       tag="bigA", bufs=1)
                    for g in range(2):
                        nc.tensor.matmul(
                            pso,
                            lhsT=ynorm[g][:, tt * 128:(tt + 1) * 128],
                            rhs=wp_sb[:, g, half * 512:(half + 1) * 512],
                            start=(g == 0), stop=(g == 1),
                        )
                    if half == 0:
                        ost_t[tt] = work.tile([128, 2, 512], BF16,
                                              name="ost", tag="ost", bufs=4)
                    ost = ost_t[tt]
                    if ring == "psS":
                        # the final drain has DVE busy; use the idle ACT
                        nc.scalar.copy(out=ost[:, half, :], in_=pso)
                    else:
                        nc.vector.tensor_copy(out=ost[:, half, :], in_=pso)
                    if half == 1:
                        nc.sync.dma_start(
                            out=outp[tt * 128:(tt + 1) * 128, :],
                            in_=ost.rearrange("p a b -> p (a b)"),
                        )
                return (700, f)

            ost_t = {}

            # ---------- schedule: one global stream ----------
            # Queue entries are (deadline, cost_ns, fn). The queue is FIFO;
            # a deadline (cq, pos) means "must be EMITTED before the S step
            # at flat position pos of pass cq" -- emitting a reader before
            # its producer would invert the dependency (Tile orders a
            # later-emitted write AFTER an earlier-emitted read).
            filler_q = []
            NODL = (99, 0)

            def grant(budget, now):
                # FIFO, but deadlines are not monotone along the queue (the
                # no-deadline projections sit in the middle), so force-pop
                # through the LAST due entry, then spend the budget.
                last_due = -1
                for i, (dl, _, _) in enumerate(filler_q):
                    if dl <= now:
                        last_due = i
                while last_due >= 0:
                    _, c, fn = filler_q.pop(0)
                    fn()
                    last_due -= 1
                    budget -= c
                while filler_q and budget > 0:
                    _, c, fn = filler_q.pop(0)
                    fn()
                    budget -= c

            def drain():
                while filler_q:
                    filler_q.pop(0)[2]()

            def enqueue_finish(cq):
                """PV + normalize + transpose for chunk cq (consumed as
                fillers of pass cq+1, or drained at the end), pair-major so
                g=0's normalize/transpose overlap g=1's PV. PV groups must
                be emitted before pass cq+1 reaches j>=4 (their eS tiles are
                then overwritten); norm/transpose before PV(cq+1) reuses the
                psY banks (FIFO order guarantees that)."""
                pv_dl = (cq + 1, pos_j4.get(cq + 1, 0))
                for g in range(2):
                    for qt in range(4):
                        for hh in range(2):
                            c, fn = pv_group(g, cq, qt, hh)
                            filler_q.append((pv_dl, c, fn))
                    for qt in range(4):
                        c, fn = norm_cl(g, cq, qt)
                        filler_q.append((pv_dl, c, fn))
                    c, fn = transp_cl(g, cq)
                    filler_q.append((pv_dl, c, fn))

            def enqueue_proj(cq, ring="bigA"):
                for qt in range(4):
                    for half in range(2):
                        c, fn = proj_cl(cq, qt, half, ring=ring)
                        filler_q.append((NODL, c, fn))

            def pass_order(cq):
                njt = 4 * cq + 4
                # smaller stagger on the last pass: g1's stream ends sooner,
                # unblocking the tail's norm/transpose/projection chain.
                lead = 3 if cq == NQ - 1 else LEAD
                order = []
                for i in range(njt + lead):
                    if i < njt:
                        order.append((0, i))
                    if i >= lead:
                        order.append((1, i - lead))
                return order

            # flat position of the first j>=4 step in each pass
            pos_j4 = {}
            for cq in range(NQ):
                for pos, (g, j) in enumerate(pass_order(cq)):
                    if j >= 4:
                        pos_j4[cq] = pos
                        break

            # prologue: all 4 QKV chains of chunk 0, c-major across four
            # parallel PSUM accumulators (bigA + both halves of two psS ring
            # slots) so each wqk/xT c-block is consumed as soon as its DMA
            # lands -- the prologue is DMA-bound, not PE-bound.
            ps_w = {}
            ps_w[0] = psp.tile([128, 512], F32, name="pw0", tag="bigA",
                               bufs=1)
            pwA = psp.tile([128, 2, 512], F32, name="pwA", tag="psS", bufs=2)
            pwB = psp.tile([128, 2, 512], F32, name="pwB", tag="psS", bufs=2)
            ps_w[2] = pwA[:, 0, :]
            ps_w[1] = pwA[:, 1, :]
            ps_w[3] = pwB[:, 0, :]
            # V tiles 0-2 accumulate c-major alongside the chains through
            # the not-yet-used psY banks and pwB's free half (each
            # accumulator needs its own bank: a group's start clears the
            # whole bank's has_written bits). Tile 3 goes through the
            # regular filler path.
            pv0 = psp.tile([128, 2, 4, HD], F32, name="pv0", tag="psY0",
                           bufs=1)
            pv1 = psp.tile([128, 2, 4, HD], F32, name="pv1", tag="psY1",
                           bufs=1)
            ps_v = {
                0: pv0.rearrange("p a b d -> p (a b d)")[:, 0:256],
                1: pv1.rearrange("p a b d -> p (a b d)")[:, 0:256],
                2: pwB[:, 1, 0:256],
            }
            for c in range(C // 128):
                for w in (0, 2, 1, 3):
                    nc.tensor.matmul(
                        ps_w[w],
                        lhsT=wqk_sb[:, c, w * 128:(w + 1) * 128],
                        rhs=xT_sb[:, c, 0:512],
                        start=(c == 0), stop=(c == C // 128 - 1),
                    )
                for tt in range(3):
                    nc.tensor.matmul(
                        ps_v[tt],
                        lhsT=xT_sb[:, c, tt * 128:(tt + 1) * 128],
                        rhs=wv_sb[:, c, :],
                        start=(c == 0), stop=(c == C // 128 - 1),
                    )
            for tt in range(3):
                nc.vector.tensor_copy(
                    out=vtil[:, tt, :, 0:HD],
                    in_=ps_v[tt].rearrange("p (h d) -> p h d", h=4),
                )
            raws0 = {}
            for w in (0, 2, 1, 3):
                raws0[w] = work.tile([128, 512], BF16, name="raw",
                                     tag="raw", bufs=4)
                nc.vector.tensor_copy(out=raws0[w], in_=ps_w[w])
            rotps = {}
            rpA = psp.tile([128, 2, 512], F32, name="rpA", tag="psS", bufs=2)
            rpB = psp.tile([128, 2, 512], F32, name="rpB", tag="psS", bufs=2)
            rotps[0], rotps[2] = rpA[:, 0, :], rpA[:, 1, :]
            rotps[1], rotps[3] = rpB[:, 0, :], rpB[:, 1, :]
            for w in (0, 2, 1, 3):
                nc.tensor.matmul(rotps[w], lhsT=rot_sb, rhs=raws0[w],
                                 start=True, stop=True)
                tmp = work.tile([128, 512], F32, name="tmp", tag="tmp",
                                bufs=4)
                nc.vector.tensor_mul(tmp, rotps[w], sin2_sb[:, 0:512])
                cosq = work.tile([128, 512], BF16, name="cosq", tag="cosq",
                                 bufs=4)
                nc.gpsimd.tensor_mul(cosq, raws0[w], cos2_sb[:, 0:512])
                nc.gpsimd.tensor_add(qk_packed[w][:, 0:512], cosq, tmp)

            for cq in range(NQ):
                if cq + 1 < NQ:
                    # chains for the next chunk must be emitted before that
                    # pass's first S step reads qk_packed.
                    for w in (0, 2, 1, 3):
                        for c, fn in chain_closures(w, cq + 1):
                            filler_q.append(((cq + 1, 0), c, fn))
                # V(cq) feeds PV(cq), whose deadline is pass cq+1 j>=4.
                # Tiles 0-2 of chunk 0 were computed in the prologue.
                for tt in range(4 * cq if cq > 0 else 3, 4 * cq + 4):
                    for c, fn in v_closures(tt):
                        filler_q.append(((cq + 1, 0), c, fn))
                if cq >= 2:
                    # defer projections of early chunks into late (ACT-heavy)
                    # passes to balance engine load.
                    enqueue_proj(cq - 2)
                    if cq == NQ - 1:
                        enqueue_proj(cq - 1)

                # finish-work unlocks inline: step (g, 4cq+qt) is the
                # last eS tile group (g, qt) needs, so its PV can enter the
                # queue immediately (pair-major: norms/transpose follow the
                # pair's last group).
                pv_dl = (cq + 1, pos_j4.get(cq + 1, 0))
                # pass 0 has few steps but a deep queue (all of chunk 1's
                # chains): grant more per step so their RoPE epilogues
                # finish before pass 1 needs qk_packed.
                step_budget = FILLER_NS * (2 if cq == 0 else 1)
                for pos, (g, j) in enumerate(pass_order(cq)):
                    grant(0, (cq, pos))
                    s_step(g, cq, j)
                    if j >= 4 * cq:
                        qt = j - 4 * cq
                        block = []
                        for hh in range(2):
                            c, fn = pv_group(g, cq, qt, hh)
                            block.append((pv_dl, c, fn))
                        if qt == 3:
                            for qt2 in range(4):
                                c, fn = norm_cl(g, cq, qt2)
                                block.append((pv_dl, c, fn))
                            c, fn = transp_cl(g, cq)
                            block.append((pv_dl, c, fn))
                        filler_q.extend(block)
                    grant(step_budget, (cq, pos))

            # epilogue: the last chunk's finish entries are already queued;
            # add its projections (the psS ring is dead after the last S
            # step, so alternate accumulators to pipeline matmul->copy->DMA)
            # and flush.
            for qt in range(4):
                for half in range(2):
                    c, fn = proj_cl(NQ - 1, qt, half,
                                    ring="psS" if half else "bigA")
                    filler_q.append((NODL, c, fn))
            drain()
    nc.finalize()
    return nc


def _prep_in_maps(x, w_attn, w_proj):
    import ml_dtypes
    bf16 = ml_dtypes.bfloat16
    x = np.asarray(x, dtype=np.float32)
    w_attn = np.asarray(w_attn, dtype=np.float32)
    w_proj = np.asarray(w_proj, dtype=np.float32)

    cos2, sin2 = _rope_tables_T()
    cossin = np.ascontiguousarray(
        np.stack([cos2, sin2], axis=1)).astype(bf16)   # [128, 2, T]
    consts3 = np.ascontiguousarray(np.stack(
        [_rot_matrix().astype(bf16),
         np.eye(128, dtype=bf16),
         _mask_tile()], axis=1))                        # [128, 3, 128]

    xTs = [np.ascontiguousarray(x[b].T).astype(bf16) for b in range(B)]
    in_maps = []
    for core in range(N_CORES):
        b = core // 4
        hbase = (core % 4) * HPC
        # wqk columns: [q_h0|q_h1, q_h2|q_h3, k_h0|k_h1, k_h2|k_h3]
        qcols = w_attn[:, hbase * HD:(hbase + HPC) * HD]
        kcols = w_attn[:, C + hbase * HD:C + (hbase + HPC) * HD]
        vcols = w_attn[:, 2 * C + hbase * HD:2 * C + (hbase + HPC) * HD]
        xwp = np.ascontiguousarray(np.concatenate(
            [np.concatenate([qcols, kcols], axis=1).astype(np.float32),
             x[b].T], axis=1)).astype(bf16)
        wv = np.ascontiguousarray(vcols).astype(bf16)
        wp = np.ascontiguousarray(
            w_proj[hbase * HD:(hbase + HPC) * HD, :]).astype(bf16)
        in_maps.append({
            "xw": xwp,
            "wv": wv,
            "wp": wp,
            "cossin": cossin,
            "consts3": consts3,
        })
    return in_maps


def _get_runner():
    """Build the SPMD jitted callable once and cache it (mirrors
    bass2jax.run_bass_via_pjrt, but reusable across kernel() calls)."""
    if "runner" in _CACHE:
        return _CACHE["runner"]

    import jax
    from jax.sharding import Mesh, PartitionSpec
    try:
        from jax.experimental.shard_map import shard_map
    except ImportError:
        from jax.shard_map import shard_map  # newer jax
    import concourse.mybir as _mybir
    from concourse import bass2jax

    nc = build_nc()
    _CACHE["nc"] = nc
    bass2jax.install_neuronx_cc_hook()

    partition_name = (
        nc.partition_id_tensor.name if nc.partition_id_tensor else None
    )
    in_names, out_names, out_avals, zero_outs = [], [], [], []
    for alloc in nc.m.functions[0].allocations:
        if not isinstance(alloc, _mybir.MemoryLocationSet):
            continue
        name = alloc.memorylocations[0].name
        if alloc.kind == "ExternalInput":
            if name != partition_name:
                in_names.append(name)
        elif alloc.kind == "ExternalOutput":
            shape = tuple(alloc.tensor_shape)
            dtype = _mybir.dt.np(alloc.dtype)
            out_names.append(name)
            out_avals.append(jax.core.ShapedArray(shape, dtype))
            zero_outs.append(np.zeros(shape, dtype))
    n_params = len(in_names)
    all_names = list(in_names) + list(out_names)
    if partition_name is not None:
        all_names.append(partition_name)
    donate = tuple(range(n_params, n_params + len(out_names)))

    def _body(*args):
        operands = list(args)
        if partition_name is not None:
            operands.append(bass2jax.partition_id_tensor())
        outs = bass2jax._bass_exec_p.bind(
            *operands,
            out_avals=tuple(out_avals),
            in_names=tuple(all_names),
            out_names=tuple(out_names),
            lowering_input_output_aliases=(),
            sim_require_finite=True,
            sim_require_nnan=True,
            nc=nc,
        )
        return tuple(outs)

    devices = jax.devices()[:N_CORES]
    mesh = Mesh(np.asarray(devices), ("core",))
    in_specs = (PartitionSpec("core"),) * (n_params + len(out_names))
    out_specs = (PartitionSpec("core"),) * len(out_names)
    sharded = jax.jit(
        shard_map(_body, mesh=mesh, in_specs=in_specs, out_specs=out_specs,
                  check_rep=False),
        donate_argnums=donate,
        keep_unused=True,
    )

    def run(in_maps):
        concat_in = [
            np.concatenate([np.asarray(in_maps[c][nm]) for c in range(N_CORES)],
                           axis=0)
            for nm in in_names
        ]
        concat_zeros = [
            np.zeros((N_CORES * z.shape[0], *z.shape[1:]), z.dtype)
            for z in zero_outs
        ]
        out_arrs = sharded(*concat_in, *concat_zeros)
        return [
            {
                nm: np.asarray(out_arrs[i]).reshape(
                    N_CORES, *out_avals[i].shape)[c]
                for i, nm in enumerate(out_names)
            }
            for c in range(N_CORES)
        ]

    _CACHE["runner"] = run
    return run


def kernel(x, w_attn, w_proj, n_head):
    assert int(n_head) == NH
    x = np.asarray(x, dtype=np.float32)
    assert x.shape == (B, T, C), x.shape

    in_maps = _prep_in_maps(x, np.asarray(w_attn), np.asarray(w_proj))
    if _CACHE.get("use_fallback"):
        results = _run_fallback(in_maps)
    else:
        try:
            run = _get_runner()
            results = run(in_maps)
        except Exception:
            _CACHE["use_fallback"] = True
            results = _run_fallback(in_maps)
    out = np.zeros((B, T, C), dtype=np.float32)
    for core in range(N_CORES):
        out[core // 4] += results[core]["outp"]
    return out


def _run_fallback(in_maps):
    """Native-NRT path (run_bass_kernel_spmd) for non-axon hosts."""
    if "nc" not in _CACHE:
        _CACHE["nc"] = build_nc()
    res = run_bass_kernel_spmd(_CACHE["nc"], in_maps,
                               core_ids=list(range(N_CORES)))
    return res.results


if __name__ == "__main__":
    rng = np.random.default_rng(0)
    x = rng.standard_normal((B, T, C)).astype(np.float32)
    wa = (rng.standard_normal((C, 3 * C)) / np.sqrt(C)).astype(np.float32)
    wpj = (rng.standard_normal((C, C)) / np.sqrt(C)).astype(np.float32)
    y = kernel(x, wa, wpj, NH)
    print("kernel ran, out:", y.shape, y.dtype, float(np.abs(y).mean()))


# revision 70
# speedup vs baseline: 1.0068x; 1.0068x over previous
"""Causal self-attention with RoPE on 8 Trainium2 NeuronCores (Bass/Tile).

Sharding: 8 cores = 2 batch elements x 4 head-groups (4 heads each), no
collectives. Each core computes QKV for its heads from a host-pretransposed
x^T, applies RoPE, runs causally-trimmed flash-style attention, and emits a
partial output projection against its w_proj row-slice; the host sums 4
partials per batch element.

v3 design (from TimelineSim engine-balance analysis):
- The exp stream on ACT (~1040ns/step) is the schedule backbone; all other
  PE work (QKV chains of the next chunk, V tiles, PV groups of the previous
  chunk, transposes, projection tiles) is emitted as paced "fillers" between
  S steps so PE never drains while ACT is the local rate limiter.
- PV uses the [q-tile, hd] layout: lhsT = eS^T tile (k x q), rhs = v (k x
  hd): N=65-ish per (q-tile, k-tile) at full 128x128 array utilization --
  half the cycles of the [hd, q] layout. Softmax denominators become
  per-partition scalars (ones-column matmuls into a dedicated PSUM bank),
  normalization is a DVE reciprocal + broadcast multiply, and a per-chunk
  PE transpose (through the psY bank's dead window) restores the [hd, t]
  layout the projection needs.
- Causal masking: exp of the diagonal tile is multiplied by a 0/1 triangle
  mask on DVE (no PE mask matmuls).
- PSUM (8 banks): S ring 2x2 + psY 2x1 + denom 1 + QKV/proj shared slot 1.
- eS tiles are materialized in SBUF per chunk (j<4 double-buffered) so the
  PV of chunk cq overlaps the S stream of chunk cq+1.
"""
import os

import numpy as np

import concourse.bass as bass
import concourse.mybir as mybir
import concourse.tile as tile
from concourse import bacc
from concourse.bass_utils import run_bass_kernel_spmd

# Problem shape (hardcoded per harness contract).
B, T, C, NH = 2, 2048, 1024, 16
HD = C // NH          # 64
HPC = NH // 4         # 4 heads per core
N_CORES = 8
ROPE_BASE = 10000.0
NEG = -1.0e30

F32 = mybir.dt.float32
BF16 = mybir.dt.bfloat16

_CACHE = {}


def _rope_tables_T():
    """cos/sin tables transposed to [HD, T], duplicated to 128 partitions
    (two 64-row head blocks). The rotate-half sign lives in the rot
    matrix, not the sin table."""
    inv_freq = 1.0 / (ROPE_BASE ** (np.arange(0, HD, 2, dtype=np.float32) / HD))
    t = np.arange(T, dtype=np.float32)
    freqs = np.outer(t, inv_freq).astype(np.float32)      # [T, 32]
    emb = np.concatenate([freqs, freqs], axis=-1)         # [T, 64]
    cosT = np.cos(emb).T.astype(np.float32)               # [64, T]
    sinT = np.sin(emb).T.astype(np.float32)
    cos2 = np.concatenate([cosT, cosT], axis=0)           # [128, T]
    sin2 = np.concatenate([sinT, sinT], axis=0)
    return np.ascontiguousarray(cos2), np.ascontiguousarray(sin2)


def _rot_matrix():
    """rot128 so that (rot128.T @ qT) = rotate_half(q)^T per 64-row head
    block: out[d] = -in[d+32] for d<32, in[d-32] for d>=32."""
    r = np.zeros((64, 64), dtype=np.float32)
    for d in range(32):
        r[d, d + 32] = -1.0
        r[d + 32, d] = 1.0
    z = np.zeros_like(r)
    rot = np.block([[r, z], [z, r]])          # [128, 128]
    return np.ascontiguousarray(rot.T)        # lhsT layout


def _mask_tile():
    """mask01 [128, 128] (bf16): 1.0 where q_local >= k_local else 0.0.
    Applied multiplicatively to exp(S) diagonal tiles on DVE."""
    import ml_dtypes
    k_l = np.arange(128)[:, None]
    q_l = np.arange(128)[None, :]
    m = np.where(q_l >= k_l, 1.0, 0.0).astype(ml_dtypes.bfloat16)
    return m


def build_nc():
    nc = bacc.Bacc(None, target_bir_lowering=False)

    # xw packs [wqk | xT] per row so one DMA fetches a c-block of both
    xw = nc.dram_tensor("xw", [C, 8 * HD + T], BF16, kind="ExternalInput")
    wv = nc.dram_tensor("wv", [C, 4 * HD], BF16, kind="ExternalInput")
    wp = nc.dram_tensor("wp", [4 * HD, C], BF16, kind="ExternalInput")
    cossin_d = nc.dram_tensor("cossin", [128, 2, T], BF16,
                              kind="ExternalInput")
    consts3_d = nc.dram_tensor("consts3", [128, 3, 128], BF16,
                               kind="ExternalInput")
    outp = nc.dram_tensor("outp", [T, C], BF16, kind="ExternalOutput")

    NT = T // 128    # 16 k-tiles
    NQ = T // 512    # 4 q-chunks
    FILLER_NS = int(os.environ.get("KERNEL_FILLER_NS", "650"))
    LEAD = int(os.environ.get("KERNEL_LEAD", "6"))

    with tile.TileContext(nc) as tc:
        with (
            tc.tile_pool(name="persist", bufs=1) as persist,
            tc.tile_pool(name="consts", bufs=1) as consts,
            tc.tile_pool(name="psp", bufs=2, space="PSUM") as psp,
            tc.tile_pool(name="work", bufs=3) as work,
        ):
            # ---- persistent tiles ----
            qk_packed = [
                persist.tile([128, T], BF16, name=f"qkp{w}", tag=f"qkp{w}")
                for w in range(4)
            ]
            vtil = persist.tile([128, NT, 4, HD + 1], BF16, name="vtil")
            ynorm = [
                persist.tile([128, T], BF16, name=f"ynorm{g}", tag=f"ynorm{g}")
                for g in range(2)
            ]
            es_main = [
                persist.tile([128, NT, 2, 512], BF16, name=f"esm{g}",
                             tag=f"esm{g}")
                for g in range(2)
            ]
            es_alt = [
                persist.tile([128, 4, 2, 512], BF16, name=f"esa{g}",
                             tag=f"esa{g}")
                for g in range(2)
            ]
            consts3_sb = consts.tile([128, 3, 128], BF16, name="consts3_sb")
            rot_sb = consts3_sb[:, 0, :]
            ident_sb = consts3_sb[:, 1, :]
            mask01_sb = consts3_sb[:, 2, :]
            wp_sb = consts.tile([128, 2, C], BF16, name="wp_sb")
            xw_sb = persist.tile([128, C // 128, 512 + T], BF16,
                                 name="xw_sb")
            wqk_sb = xw_sb[:, :, 0:512]
            xT_sb = xw_sb[:, :, 512:512 + T]
            cossin_sb = persist.tile([128, 2, T], BF16, name="cossin_sb")
            cos2_sb = cossin_sb[:, 0, :]
            sin2_sb = cossin_sb[:, 1, :]
            wv_sb = persist.tile([128, C // 128, 4 * HD], BF16, name="wv_sb")

            xw_r = xw.rearrange("(co p) t -> p co t", p=128)

            # Each DMA's issue holds HWDGE ~625ns exclusively, and small
            # transfers don't hide the next issue, so the head is
            # issue-rate-bound: use few, large DMAs ordered by first
            # consumption (tables are host-packed into single tensors).
            for c in range(2):
                nc.sync.dma_start(out=xw_sb[:, c:c + 1, 0:1024],
                                  in_=xw_r[:, c:c + 1, 0:1024])
            nc.sync.dma_start(out=consts3_sb, in_=consts3_d[:, :, :])
            nc.sync.dma_start(out=xw_sb[:, 2:5, 0:1024],
                              in_=xw_r[:, 2:5, 0:1024])
            nc.sync.dma_start(out=xw_sb[:, 5:8, 0:1024],
                              in_=xw_r[:, 5:8, 0:1024])
            nc.sync.dma_start(out=cossin_sb[:, :, 0:512],
                              in_=cossin_d[:, :, 0:512])
            nc.sync.dma_start(
                out=wv_sb, in_=wv.rearrange("(co p) n -> p co n", p=128))
            nc.sync.dma_start(out=xw_sb[:, :, 1024:1536],
                              in_=xw_r[:, :, 1024:1536])
            nc.sync.dma_start(out=cossin_sb[:, :, 512:1024],
                              in_=cossin_d[:, :, 512:1024])
            nc.sync.dma_start(out=xw_sb[:, :, 1536:2048],
                              in_=xw_r[:, :, 1536:2048])
            nc.sync.dma_start(out=cossin_sb[:, :, 1024:T],
                              in_=cossin_d[:, :, 1024:T])
            nc.sync.dma_start(
                out=wp_sb, in_=wp.rearrange("(gg p) n -> p gg n", p=128))
            nc.sync.dma_start(out=xw_sb[:, :, 2048:2560],
                              in_=xw_r[:, :, 2048:2560])

            ones64 = persist.tile([128, NT * 4], BF16, name="ones64")
            nc.vector.memset(ones64, 1.0)
            # tiny warm-up matmul so the PE p-state ramp starts during the
            # head DMA wait instead of on the first real matmul
            warm = psp.tile([64, 64], F32, name="warm", tag="bigA", bufs=1)
            nc.tensor.matmul(warm, lhsT=ones64[:, 0:64], rhs=ones64[:, 0:64],
                             start=True, stop=True)
            nc.vector.tensor_copy(
                out=vtil[:, :, :, HD:HD + 1],
                in_=ones64.rearrange("p (a b) -> p a b", a=NT).unsqueeze(-1),
            )
            inv_sqrt_hd = float(1.0 / np.sqrt(HD))

            def es_ap(g, cq, j):
                """eS store for (chunk, k-tile): j<4 double-buffered by
                chunk parity so S(cq+1) can overlap PV(cq)."""
                if j < 4 and (cq % 2) == 1:
                    return es_alt[g][:, j]
                return es_main[g][:, j]

            # ---------- emission pieces ----------
            def chain_closures(w, cq):
                """QKV chain for wqk column-block w over chunk cq + RoPE.
                Returns (pe_cost_ns, closure) list; the accumulator lives in
                the shared 1-bank bigA slot, rotate-half reuses it after the
                raw copy frees it."""
                sl = slice(cq * 512, (cq + 1) * 512)
                st = {}
                out = []

                def mk(c):
                    def f():
                        if c == 0:
                            st["ps"] = psp.tile([128, 512], F32, name="psqkv",
                                                tag="bigA", bufs=1)
                        nc.tensor.matmul(
                            st["ps"],
                            lhsT=wqk_sb[:, c, w * 128:(w + 1) * 128],
                            rhs=xT_sb[:, c, sl],
                            start=(c == 0), stop=(c == C // 128 - 1),
                        )
                    return f
                for c in range(C // 128):
                    out.append((220, mk(c)))

                def raw_f():
                    raw = work.tile([128, 512], BF16, name="raw", tag="raw",
                                    bufs=4)
                    st["raw"] = raw
                    nc.vector.tensor_copy(out=raw, in_=st["ps"])
                out.append((0, raw_f))

                def rot_f():
                    psr = psp.tile([128, 512], F32, name="psr",
                                   tag="bigA", bufs=1)
                    nc.tensor.matmul(psr, lhsT=rot_sb, rhs=st["raw"],
                                     start=True, stop=True)
                    tmp = work.tile([128, 512], F32, name="tmp", tag="tmp",
                                    bufs=4)
                    nc.vector.tensor_mul(tmp, psr, sin2_sb[:, sl])
                    cosq = work.tile([128, 512], BF16, name="cosq",
                                     tag="cosq", bufs=4)
                    if w in (0, 2):
                        # g=0's qk blocks gate the next pass's first S
                        # steps; keep their epilogue off the slow GPSIMD.
                        nc.vector.tensor_mul(cosq, st["raw"], cos2_sb[:, sl])
                        nc.vector.tensor_add(qk_packed[w][:, sl], cosq, tmp)
                    else:
                        nc.gpsimd.tensor_mul(cosq, st["raw"], cos2_sb[:, sl])
                        nc.gpsimd.tensor_add(qk_packed[w][:, sl], cosq, tmp)
                out.append((220, rot_f))
                return out

            def v_closures(tt):
                """V tile tt (t-major) through the shared bigA slot."""
                st = {}
                out = []

                def mk(c):
                    def f():
                        if c == 0:
                            st["ps"] = psp.tile([128, 4 * HD], F32,
                                                name="psv", tag="bigA",
                                                bufs=1)
                        nc.tensor.matmul(
                            st["ps"],
                            lhsT=xT_sb[:, c, tt * 128:(tt + 1) * 128],
                            rhs=wv_sb[:, c, :],
                            start=(c == 0), stop=(c == C // 128 - 1),
                        )
                    return f
                for c in range(C // 128):
                    out.append((110, mk(c)))

                def epi():
                    nc.vector.tensor_copy(
                        out=vtil[:, tt, :, 0:HD],
                        in_=st["ps"].rearrange("p (h d) -> p h d", h=4),
                    )
                out.append((0, epi))
                return out

            def s_step(g, cq, j):
                """S^T matmuls (both heads of pair g) for k-tile j, exp on
                ACT into the eS store, 0/1 mask multiply on DVE for the
                diagonal tile."""
                off = max(0, (j - 4 * cq) * 128)
                F = 512 - off
                qlo = cq * 512 + off
                psS = psp.tile([128, 2, 512], F32, name="psS", tag="psS",
                               bufs=2)
                for hh in range(2):
                    poff = 64 * hh
                    nc.tensor.matmul(
                        psS[:, hh, 0:F],
                        lhsT=qk_packed[2 + g][poff:poff + 64,
                                              j * 128:(j + 1) * 128],
                        rhs=qk_packed[g][poff:poff + 64, qlo:qlo + F],
                        start=True, stop=True,
                    )
                est = es_ap(g, cq, j)
                nc.scalar.activation(
                    out=est[:, :, off:512], in_=psS[:, :, 0:F],
                    func=mybir.ActivationFunctionType.Exp,
                    scale=inv_sqrt_hd,
                )
                if j >= 4 * cq:
                    nc.vector.tensor_tensor(
                        out=est[:, :, off:off + 128],
                        in0=est[:, :, off:off + 128],
                        in1=mask01_sb.unsqueeze(1).to_broadcast([128, 2, 128]),
                        op=mybir.AluOpType.mult,
                    )

            psY_t = {}
            psD_t = {}

            def pv_group(g, cq, qt, hh):
                """One PV accumulation group: q-tile qt of pair-g head hh,
                contracting over k-tiles 0..4cq+qt into psY[q, hd] plus the
                ones-column denominator into the psD bank."""
                njt = 4 * cq + qt + 1

                def f():
                    if psY_t.get(g, (None, -1))[1] != cq:
                        psY_t[g] = (psp.tile([128, 2, 4, HD], F32,
                                             name=f"psY{g}", tag=f"psY{g}",
                                             bufs=1), cq)
                    if psD_t.get("cq", -1) != cq:
                        psD_t["t"] = psp.tile([128, 2, 2, 4, 1], F32,
                                              name="psD", tag="psD", bufs=1)
                        psD_t["cq"] = cq
                    psY = psY_t[g][0]
                    psD = psD_t["t"]
                    h = 2 * g + hh
                    for j in range(njt):
                        lhsT = es_ap(g, cq, j)[:, hh, qt * 128:(qt + 1) * 128]
                        nc.tensor.matmul(
                            psY[:, hh, qt, :], lhsT=lhsT,
                            rhs=vtil[:, j, h, 0:HD],
                            start=(j == 0), stop=(j == njt - 1),
                        )
                        nc.tensor.matmul(
                            psD[:, g, hh, qt, :], lhsT=lhsT,
                            rhs=vtil[:, j, h, HD:HD + 1],
                            start=(j == 0), stop=(j == njt - 1),
                        )
                return (njt * 2 * 30, f)

            ysb_t = {}

            def norm_cl(g, cq, qt):
                """Normalize q-tile qt of pair g: per-partition reciprocal
                of the denominator column, broadcast multiply out of PSUM."""
                def f():
                    psY = psY_t[g][0]
                    psD = psD_t["t"]
                    rec = work.tile([128, 2, 1], F32, name="rec", tag="rec",
                                    bufs=6)
                    nc.vector.reciprocal(rec, psD[:, g, :, qt, :])
                    ysb = work.tile([128, 2, HD], BF16, name="ysb",
                                    tag="ysb", bufs=10)
                    nc.vector.tensor_tensor(
                        out=ysb, in0=psY[:, :, qt, :],
                        in1=rec.to_broadcast([128, 2, HD]),
                        op=mybir.AluOpType.mult,
                    )
                    ysb_t[(g, qt)] = ysb
                return (0, f)

            def transp_cl(g, cq):
                """Transpose the chunk's 4 normalized q-tiles of pair g back
                to [hd, t] through the psY bank's dead window, then one ACT
                copy into ynorm."""
                def f():
                    psT = psp.tile([128, 4, 128], F32, name=f"psT{g}",
                                   tag=f"psY{g}", bufs=1)
                    for qt in range(4):
                        nc.tensor.matmul(
                            psT[:, qt, :],
                            lhsT=ysb_t[(g, qt)].rearrange("p a b -> p (a b)"),
                            rhs=ident_sb,
                            start=True, stop=True,
                        )
                    if cq == NQ - 1:
                        # tail: DVE is the drain bottleneck, ACT is idle
                        nc.scalar.copy(
                            out=ynorm[g][:, cq * 512:(cq + 1) * 512],
                            in_=psT.rearrange("p a b -> p (a b)"),
                        )
                    else:
                        nc.vector.tensor_copy(
                            out=ynorm[g][:, cq * 512:(cq + 1) * 512],
                            in_=psT.rearrange("p a b -> p (a b)"),
                        )
                return (4 * 55, f)

            def proj_cl(cq, qt, half, ring="bigA"):
                """Half-width projection of t-tile 4cq+qt; accumulator from
                the shared bigA slot (or the freed psS ring in the final
                drain); PSUM->SBUF copy alternates ACT/DVE. Both halves of a
                t-tile share one ost tile and one output DMA (HWDGE issue
                costs ~625ns each). The filler cost is inflated past one
                grant so consecutive projections are spaced by at least one
                S step."""
                tt = 4 * cq + qt

                def f():
                    if ring == "psS":
                        pso = psp.tile([128, 2, 512], F32, name="psoS",
                                       tag="psS", bufs=2)[:, 0, :]
                    else:
                        pso = psp.tile([128, 512], F32, name="pso",
                                       tag="bigA", bufs=1)
                    for g in range(2):
                        nc.tensor.matmul(
                            pso,
                            lhsT=ynorm[g][:, tt * 128:(tt + 1) * 128],
                            rhs=wp_sb[:, g, half * 512:(half + 1) * 512],
                            start=(g == 0), stop=(g == 1),
                        )
                    if half == 0:
                        ost_t[tt] = work.tile([128, 2, 512], BF16,
                                              name="ost", tag="ost", bufs=4)
                    ost = ost_t[tt]
                    if ring == "psS":
                        # the final drain has DVE busy; use the idle ACT
                        nc.scalar.copy(out=ost[:, half, :], in_=pso)
                    else:
                        nc.vector.tensor_copy(out=ost[:, half, :], in_=pso)
                    if half == 1:
                        nc.sync.dma_start(
                            out=outp[tt * 128:(tt + 1) * 128, :],
                            in_=ost.rearrange("p a b -> p (a b)"),
                        )
                return (700, f)

            ost_t = {}

            # ---------- schedule: one global stream ----------
            # Queue entries are (deadline, cost_ns, fn). The queue is FIFO;
            # a deadline (cq, pos) means "must be EMITTED before the S step
            # at flat position pos of pass cq" -- emitting a reader before
            # its producer would invert the dependency (Tile orders a
            # later-emitted write AFTER an earlier-emitted read).
            filler_q = []
            NODL = (99, 0)

            def grant(budget, now):
                # FIFO, but deadlines are not monotone along the queue (the
                # no-deadline projections sit in the middle), so force-pop
                # through the LAST due entry, then spend the budget.
                last_due = -1
                for i, (dl, _, _) in enumerate(filler_q):
                    if dl <= now:
                        last_due = i
                while last_due >= 0:
                    _, c, fn = filler_q.pop(0)
                    fn()
                    last_due -= 1
                    budget -= c
                while filler_q and budget > 0:
                    _, c, fn = filler_q.pop(0)
                    fn()
                    budget -= c

            def drain():
                while filler_q:
                    filler_q.pop(0)[2]()

            def enqueue_finish(cq):
                """PV + normalize + transpose for chunk cq (consumed as
                fillers of pass cq+1, or drained at the end), pair-major so
                g=0's normalize/transpose overlap g=1's PV. PV groups must
                be emitted before pass cq+1 reaches j>=4 (their eS tiles are
                then overwritten); norm/transpose before PV(cq+1) reuses the
                psY banks (FIFO order guarantees that)."""
                pv_dl = (cq + 1, pos_j4.get(cq + 1, 0))
                for g in range(2):
                    for qt in range(4):
                        for hh in range(2):
                            c, fn = pv_group(g, cq, qt, hh)
                            filler_q.append((pv_dl, c, fn))
                    for qt in range(4):
                        c, fn = norm_cl(g, cq, qt)
                        filler_q.append((pv_dl, c, fn))
                    c, fn = transp_cl(g, cq)
                    filler_q.append((pv_dl, c, fn))

            def enqueue_proj(cq, ring="bigA"):
                for qt in range(4):
                    for half in range(2):
                        c, fn = proj_cl(cq, qt, half, ring=ring)
                        filler_q.append((NODL, c, fn))

            def pass_order(cq):
                njt = 4 * cq + 4
                # smaller stagger on the last pass: g1's stream ends sooner,
                # unblocking the tail's norm/transpose/projection chain.
                lead = 3 if cq == NQ - 1 else LEAD
                order = []
                for i in range(njt + lead):
                    if i < njt:
                        order.append((0, i))
                    if i >= lead:
                        order.append((1, i - lead))
                return order

            # flat position of the first j>=4 step in each pass
            pos_j4 = {}
            for cq in range(NQ):
                for pos, (g, j) in enumerate(pass_order(cq)):
                    if j >= 4:
                        pos_j4[cq] = pos
                        break

            # prologue: all 4 QKV chains of chunk 0, c-major across four
            # parallel PSUM accumulators (bigA + both halves of two psS ring
            # slots) so each wqk/xT c-block is consumed as soon as its DMA
            # lands -- the prologue is DMA-bound, not PE-bound.
            ps_w = {}
            ps_w[0] = psp.tile([128, 512], F32, name="pw0", tag="bigA",
                               bufs=1)
            pwA = psp.tile([128, 2, 512], F32, name="pwA", tag="psS", bufs=2)
            pwB = psp.tile([128, 2, 512], F32, name="pwB", tag="psS", bufs=2)
            ps_w[2] = pwA[:, 0, :]
            ps_w[1] = pwA[:, 1, :]
            ps_w[3] = pwB[:, 0, :]
            # V tiles 0-2 accumulate c-major alongside the chains through
            # the not-yet-used psY banks and pwB's free half (each
            # accumulator needs its own bank: a group's start clears the
            # whole bank's has_written bits). Tile 3 goes through the
            # regular filler path.
            pv0 = psp.tile([128, 2, 4, HD], F32, name="pv0", tag="psY0",
                           bufs=1)
            pv1 = psp.tile([128, 2, 4, HD], F32, name="pv1", tag="psY1",
                           bufs=1)
            ps_v = {
                0: pv0.rearrange("p a b d -> p (a b d)")[:, 0:256],
                1: pv1.rearrange("p a b d -> p (a b d)")[:, 0:256],
                2: pwB[:, 1, 0:256],
            }
            for c in range(C // 128):
                for w in (0, 2, 1, 3):
                    nc.tensor.matmul(
                        ps_w[w],
                        lhsT=wqk_sb[:, c, w * 128:(w + 1) * 128],
                        rhs=xT_sb[:, c, 0:512],
                        start=(c == 0), stop=(c == C // 128 - 1),
                    )
                for tt in range(3):
                    nc.tensor.matmul(
                        ps_v[tt],
                        lhsT=xT_sb[:, c, tt * 128:(tt + 1) * 128],
                        rhs=wv_sb[:, c, :],
                        start=(c == 0), stop=(c == C // 128 - 1),
                    )
            for tt in range(3):
                nc.vector.tensor_copy(
                    out=vtil[:, tt, :, 0:HD],
                    in_=ps_v[tt].rearrange("p (h d) -> p h d", h=4),
                )
            raws0 = {}
            for w in (0, 2, 1, 3):
                raws0[w] = work.tile([128, 512], BF16, name="raw",
                                     tag="raw", bufs=4)
                nc.vector.tensor_copy(out=raws0[w], in_=ps_w[w])
            rotps = {}
            rpA = psp.tile([128, 2, 512], F32, name="rpA", tag="psS", bufs=2)
            rpB = psp.tile([128, 2, 512], F32, name="rpB", tag="psS", bufs=2)
            rotps[0], rotps[2] = rpA[:, 0, :], rpA[:, 1, :]
            rotps[1], rotps[3] = rpB[:, 0, :], rpB[:, 1, :]
            for w in (0, 2, 1, 3):
                nc.tensor.matmul(rotps[w], lhsT=rot_sb, rhs=raws0[w],
                                 start=True, stop=True)
                tmp = work.tile([128, 512], F32, name="tmp", tag="tmp",
                                bufs=4)
                nc.vector.tensor_mul(tmp, rotps[w], sin2_sb[:, 0:512])
                cosq = work.tile([128, 512], BF16, name="cosq", tag="cosq",
                                 bufs=4)
                nc.gpsimd.tensor_mul(cosq, raws0[w], cos2_sb[:, 0:512])
                nc.gpsimd.tensor_add(qk_packed[w][:, 0:512], cosq, tmp)

            for cq in range(NQ):
                if cq + 1 < NQ:
                    # chains for the next chunk must be emitted before that
                    # pass's first S step reads qk_packed.
                    for w in (0, 2, 1, 3):
                        for c, fn in chain_closures(w, cq + 1):
                            filler_q.append(((cq + 1, 0), c, fn))
                # V(cq) feeds PV(cq), whose deadline is pass cq+1 j>=4.
                # Tiles 0-2 of chunk 0 were computed in the prologue.
                for tt in range(4 * cq if cq > 0 else 3, 4 * cq + 4):
                    for c, fn in v_closures(tt):
                        filler_q.append(((cq + 1, 0), c, fn))
                if cq >= 2:
                    # defer projections of early chunks into late (ACT-heavy)
                    # passes to balance engine load.
                    enqueue_proj(cq - 2)
                    if cq == NQ - 1:
                        enqueue_proj(cq - 1)

                # finish-work unlocks inline: step (g, 4cq+qt) is the
                # last eS tile group (g, qt) needs, so its PV can enter the
                # queue immediately (pair-major: norms/transpose follow the
                # pair's last group).
                pv_dl = (cq + 1, pos_j4.get(cq + 1, 0))
                # pass 0 has few steps but a deep queue (all of chunk 1's
                # chains): grant more per step so their RoPE epilogues
                # finish before pass 1 needs qk_packed.
                step_budget = FILLER_NS * (2 if cq == 0 else 1)
                for pos, (g, j) in enumerate(pass_order(cq)):
                    grant(0, (cq, pos))
                    s_step(g, cq, j)
                    if j >= 4 * cq:
                        qt = j - 4 * cq
                        block = []
                        for hh in range(2):
                            c, fn = pv_group(g, cq, qt, hh)
                            block.append((pv_dl, c, fn))
                        if qt == 3:
                            for qt2 in range(4):
                                c, fn = norm_cl(g, cq, qt2)
                                block.append((pv_dl, c, fn))
                            c, fn = transp_cl(g, cq)
                            block.append((pv_dl, c, fn))
                        filler_q.extend(block)
                    grant(step_budget, (cq, pos))

            # epilogue: the last chunk's finish entries are already queued;
            # add its projections (the psS ring is dead after the last S
            # step, so alternate accumulators to pipeline matmul->copy->DMA)
            # and flush.
            for qt in range(4):
                for half in range(2):
                    c, fn = proj_cl(NQ - 1, qt, half,
                                    ring="psS" if half else "bigA")
                    filler_q.append((NODL, c, fn))
            drain()
    nc.finalize()
    return nc


def _prep_in_maps(x, w_attn, w_proj):
    import ml_dtypes
    bf16 = ml_dtypes.bfloat16
    x = np.asarray(x, dtype=np.float32)
    w_attn = np.asarray(w_attn, dtype=np.float32)
    w_proj = np.asarray(w_proj, dtype=np.float32)

    cos2, sin2 = _rope_tables_T()
    cossin = np.ascontiguousarray(
        np.stack([cos2, sin2], axis=1)).astype(bf16)   # [128, 2, T]
    consts3 = np.ascontiguousarray(np.stack(
        [_rot_matrix().astype(bf16),
         np.eye(128, dtype=bf16),
         _mask_tile()], axis=1))                        # [128, 3, 128]

    xTs = [np.ascontiguousarray(x[b].T).astype(bf16) for b in range(B)]
    in_maps = []
    for core in range(N_CORES):
        b = core // 4
        hbase = (core % 4) * HPC
        # wqk columns: [q_h0|q_h1, q_h2|q_h3, k_h0|k_h1, k_h2|k_h3]
        qcols = w_attn[:, hbase * HD:(hbase + HPC) * HD]
        kcols = w_attn[:, C + hbase * HD:C + (hbase + HPC) * HD]
        vcols = w_attn[:, 2 * C + hbase * HD:2 * C + (hbase + HPC) * HD]
        xwp = np.ascontiguousarray(np.concatenate(
            [np.concatenate([qcols, kcols], axis=1).astype(np.float32),
             x[b].T], axis=1)).astype(bf16)
        wv = np.ascontiguousarray(vcols).astype(bf16)
        wp = np.ascontiguousarray(
            w_proj[hbase * HD:(hbase + HPC) * HD, :]).astype(bf16)
        in_maps.append({
            "xw": xwp,
            "wv": wv,
            "wp": wp,
            "cossin": cossin,
            "consts3": consts3,
        })
    return in_maps


def _get_runner():
    """Build the SPMD jitted callable once and cache it (mirrors
    bass2jax.run_bass_via_pjrt, but reusable across kernel() calls)."""
    if "runner" in _CACHE:
        return _CACHE["runner"]

    import jax
    from jax.sharding import Mesh, PartitionSpec
    try:
        from jax.experimental.shard_map import shard_map
    except ImportError:
        from jax.shard_map import shard_map  # newer jax
    import concourse.mybir as _mybir
    from concourse import bass2jax

    nc = build_nc()
    _CACHE["nc"] = nc
    bass2jax.install_neuronx_cc_hook()

    partition_name = (
        nc.partition_id_tensor.name if nc.partition_id_tensor else None
    )
    in_names, out_names, out_avals, zero_outs = [], [], [], []
    for alloc in nc.m.functions[0].allocations:
        if not isinstance(alloc, _mybir.MemoryLocationSet):
            continue
        name = alloc.memorylocations[0].name
        if alloc.kind == "ExternalInput":
            if name != partition_name:
                in_names.append(name)
        elif alloc.kind == "ExternalOutput":
            shape = tuple(alloc.tensor_shape)
            dtype = _mybir.dt.np(alloc.dtype)
            out_names.append(name)
            out_avals.append(jax.core.ShapedArray(shape, dtype))
            zero_outs.append(np.zeros(shape, dtype))
    n_params = len(in_names)
    all_names = list(in_names) + list(out_names)
    if partition_name is not None:
        all_names.append(partition_name)
    donate = tuple(range(n_params, n_params + len(out_names)))

    def _body(*args):
        operands = list(args)
        if partition_name is not None:
            operands.append(bass2jax.partition_id_tensor())
        outs = bass2jax._bass_exec_p.bind(
            *operands,
            out_avals=tuple(out_avals),
            in_names=tuple(all_names),
            out_names=tuple(out_names),
            lowering_input_output_aliases=(),
            sim_require_finite=True,
            sim_require_nnan=True,
            nc=nc,
        )
        return tuple(outs)

    devices = jax.devices()[:N_CORES]
    mesh = Mesh(np.asarray(devices), ("core",))
    in_specs = (PartitionSpec("core"),) * (n_params + len(out_names))
    out_specs = (PartitionSpec("core"),) * len(out_names)
    sharded = jax.jit(
        shard_map(_body, mesh=mesh, in_specs=in_specs, out_specs=out_specs,
                  check_rep=False),
        donate_argnums=donate,
        keep_unused=True,
    )

    def run(in_maps):
        concat_in = [
            np.concatenate([np.asarray(in_maps[c][nm]) for c in range(N_CORES)],
                           axis=0)
            for nm in in_names
        ]
        concat_zeros = [
            np.zeros((N_CORES * z.shape[0], *z.shape[1:]), z.dtype)
            for z in zero_outs
        ]
        out_arrs = sharded(*concat_in, *concat_zeros)
        return [
            {
                nm: np.asarray(out_arrs[i]).reshape(
                    N_CORES, *out_avals[i].shape)[c]
                for i, nm in enumerate(out_names)
            }
            for c in range(N_CORES)
        ]

    _CACHE["runner"] = run
    return run


def kernel(x, w_attn, w_proj, n_head):
    assert int(n_head) == NH
    x = np.asarray(x, dtype=np.float32)
    assert x.shape == (B, T, C), x.shape

    in_maps = _prep_in_maps(x, np.asarray(w_attn), np.asarray(w_proj))
    if _CACHE.get("use_fallback"):
        results = _run_fallback(in_maps)
    else:
        try:
            run = _get_runner()
            results = run(in_maps)
        except Exception:
            _CACHE["use_fallback"] = True
            results = _run_fallback(in_maps)
    out = np.zeros((B, T, C), dtype=np.float32)
    for core in range(N_CORES):
        out[core // 4] += results[core]["outp"]
    return out


def _run_fallback(in_maps):
    """Native-NRT path (run_bass_kernel_spmd) for non-axon hosts."""
    if "nc" not in _CACHE:
        _CACHE["nc"] = build_nc()
    res = run_bass_kernel_spmd(_CACHE["nc"], in_maps,
                               core_ids=list(range(N_CORES)))
    return res.results


if __name__ == "__main__":
    rng = np.random.default_rng(0)
    x = rng.standard_normal((B, T, C)).astype(np.float32)
    wa = (rng.standard_normal((C, 3 * C)) / np.sqrt(C)).astype(np.float32)
    wpj = (rng.standard_normal((C, C)) / np.sqrt(C)).astype(np.float32)
    y = kernel(x, wa, wpj, NH)
    print("kernel ran, out:", y.shape, y.dtype, float(np.abs(y).mean()))
